# revision 12
# baseline (speedup 1.0000x reference)
# Trainium2 Bass kernel for nn_Cobrablock (dense transformer block).
# Sharding: 8-way over (batch, seq-block): core c -> batch c//4, seq rows [512*(c%4), ...+512).
# Activations feature-major ([feat, row]). QKV + head projections in fp8e4 DoubleRow
# (weights prescaled by powers of 2, descaled at exp / copy points); scores bf16;
# attention output accumulated in [query-partition, head-dim] orientation with the
# softmax denominator as a 33rd ones-column; SwiGLU in bf16. One AllGather per
# 4-core batch group exchanges keyh^T / valh blocks.
import math

import numpy as np
import ml_dtypes

BF16 = ml_dtypes.bfloat16
FP8 = ml_dtypes.float8_e4m3

B, S, D = 2, 2048, 1024
H, K = 8, 32
HK = H * K  # 256
N_CORES = 8
ROWS = 512
NT = D // 128  # 8 feature tiles
EPS = 1e-5

_CACHE = {}


def _build_program(flags, trace_sim=False):
    import concourse.bass as bass
    import concourse.mybir as mybir
    import concourse.tile as tile
    from concourse import bacc

    f32 = mybir.dt.float32
    bf16 = mybir.dt.bfloat16
    fp8 = mybir.dt.float8e4
    AF = mybir.ActivationFunctionType
    OP = mybir.AluOpType
    DR = mybir.MatmulPerfMode.DoubleRow

    nc = bacc.Bacc("TRN2", target_bir_lowering=False, debug=False, num_devices=N_CORES)

    xT = nc.dram_tensor("xT", [D, ROWS], bf16, kind="ExternalInput")
    w8 = {}
    for nm in ("w1", "w2", "w3"):
        w8[nm] = nc.dram_tensor(nm, [D, D], fp8, kind="ExternalInput")
    wbig = {}
    for nm in ("wg", "wu", "wd"):
        wbig[nm] = nc.dram_tensor(nm, [D, D], bf16, kind="ExternalInput")
    wq = nc.dram_tensor("wq", [D, HK], fp8, kind="ExternalInput")
    wk = nc.dram_tensor("wk", [D, HK], fp8, kind="ExternalInput")
    wv = nc.dram_tensor("wv", [D, HK], fp8, kind="ExternalInput")
    wo = nc.dram_tensor("wo", [HK, D], bf16, kind="ExternalInput")
    bcols = nc.dram_tensor("bcols", [10, D], f32, kind="ExternalInput")
    cossin = nc.dram_tensor("cossin", [128, 4, 2, ROWS], fp8, kind="ExternalInput")
    identD = nc.dram_tensor("identD", [128, 128], bf16, kind="ExternalInput")
    outT = nc.dram_tensor("outT", [D, ROWS], bf16, kind="ExternalOutput")

    EXP_SCALE = 2.0 ** -22

    with tile.TileContext(nc, trace_sim=trace_sim) as tc:
        from contextlib import ExitStack

        ctx = ExitStack()
        const = ctx.enter_context(tc.tile_pool(name="const", bufs=1))
        wts = ctx.enter_context(tc.tile_pool(name="wts", bufs=3))
        wbg = ctx.enter_context(tc.tile_pool(name="wbg", bufs=2))
        wsm = ctx.enter_context(tc.tile_pool(name="wsm", bufs=1))
        tmp = ctx.enter_context(tc.tile_pool(name="tmp", bufs=6))
        rows = ctx.enter_context(tc.tile_pool(name="rows", bufs=5))
        bcp = ctx.enter_context(tc.tile_pool(name="bcp", bufs=2))
        aonp = ctx.enter_context(tc.tile_pool(name="aonp", bufs=4))
        epool = ctx.enter_context(tc.tile_pool(name="epool", bufs=33))
        acts = ctx.enter_context(tc.tile_pool(name="acts", bufs=4))
        dram = ctx.enter_context(tc.tile_pool(name="dram", bufs=1, space="DRAM"))

        # ---- constants / inputs resident in SBUF ----
        bias_sb = const.tile([128, 10, 8], f32)
        nc.sync.dma_start(bias_sb[:], bcols.rearrange("r (o p) -> p r o", p=128))
        bhc = const.tile([128, 6], f32)  # head-bias cols: q:0,1 k:2,3 v:4,5
        nc.sync.dma_start(bhc[:], bcols[8:9, 0:768].rearrange("r (t o p) -> p (r t o)", p=128, t=3, o=2))
        bvb = const.tile([128, HK], f32)  # v-head bias broadcast over partitions
        nc.sync.dma_start(bvb[:], bcols[8:9, 512:768].to_broadcast([128, HK]))
        cs_sb = const.tile([128, 4, 2, ROWS], fp8)
        nc.sync.dma_start(cs_sb[:], cossin[:, :, :, :])
        ident = const.tile([128, 128], bf16)
        nc.sync.dma_start(ident[:], identD[:, :])
        eps_sb = const.tile([1, 1], f32)
        nc.vector.memset(eps_sb[:], EPS)
        ones128 = const.tile([1, 128], f32)
        nc.vector.memset(ones128[:], 1.0)
        invd = const.tile([128, 1], bf16)
        nc.vector.memset(invd[:], 1.0 / D)
        xT_sb = const.tile([128, NT, ROWS], bf16)
        nc.sync.dma_start(xT_sb[:], xT.rearrange("(o p) f -> p o f", p=128))

        def col(r, kt):
            return bias_sb[:, r, kt:kt + 1]

        w_sb = {}
        for nm in ("w2", "w3", "w1"):
            w_sb[nm] = wts.tile([128, NT, D], fp8, tag="W8", name=f"W_{nm}")
            nc.sync.dma_start(w_sb[nm][:], w8[nm].rearrange("(o p) c -> p o c", p=128))
        wq_sb = wsm.tile([128, NT, HK], fp8)
        nc.sync.dma_start(wq_sb[:], wq.rearrange("(o p) c -> p o c", p=128))
        wk_sb = wsm.tile([128, NT, HK], fp8)
        nc.sync.dma_start(wk_sb[:], wk.rearrange("(o p) c -> p o c", p=128))
        wv_sb = wsm.tile([128, NT, HK], fp8)
        nc.sync.dma_start(wv_sb[:], wv.rearrange("(o p) c -> p o c", p=128))
        wo_sb = wsm.tile([128, 2, D], bf16)
        nc.sync.dma_start(wo_sb[:], wo.rearrange("(o p) c -> p o c", p=128))

        h_sb = acts.tile([128, NT, ROWS], fp8, tag="act8", name="h_sb")

        # ================= LN1 (x already bf16; ln1 g/b folded into w1-3) ========
        with tc.tile_pool(name="ps_st", bufs=2, space="PSUM") as ps_st:
            sum_ps = ps_st.tile([1, ROWS], f32, tag="st", name="sum_ps")
            sumsq_ps = ps_st.tile([1, ROWS], f32, tag="st", name="sumsq_ps")
            for kt in range(NT):
                sq = tmp.tile([128, ROWS], bf16, tag="t2", name="sq")
                nc.vector.tensor_tensor(sq[:], xT_sb[:, kt], xT_sb[:, kt], OP.mult)
                nc.tensor.matmul(sum_ps[:], invd[:], xT_sb[:, kt], start=(kt == 0), stop=(kt == NT - 1))
                nc.tensor.matmul(sumsq_ps[:], invd[:], sq[:], start=(kt == 0), stop=(kt == NT - 1))
            mean = rows.tile([1, ROWS], f32, tag="row", name="mean")
            nc.vector.tensor_copy(out=mean[:], in_=sum_ps[:])
            msq = rows.tile([1, ROWS], f32, tag="row", name="msq")
            nc.vector.tensor_tensor(msq[:], mean[:], mean[:], OP.mult)
            var = rows.tile([1, ROWS], f32, tag="row", name="var")
            nc.vector.tensor_tensor(var[:], sumsq_ps[:], msq[:], OP.subtract)
            std = rows.tile([1, ROWS], f32, tag="row", name="std")
            nc.scalar.activation(std[:], var[:], AF.Sqrt, bias=eps_sb[:])
            rstd = rows.tile([1, ROWS], f32, tag="row", name="rstd")
            nc.vector.reciprocal(rstd[:], std[:])
            m_ps = ps_st.tile([128, ROWS], f32, tag="st", name="m_ps")
            nc.tensor.matmul(m_ps[:], ones128[:], mean[:], start=True, stop=True)
            r_ps = ps_st.tile([128, ROWS], f32, tag="st", name="r_ps")
            nc.tensor.matmul(r_ps[:], ones128[:], rstd[:], start=True, stop=True)
            m_bf = bcp.tile([128, ROWS], bf16, tag="bc", name="m_bf")
            nc.scalar.activation(m_bf[:], m_ps[:], AF.Copy)
            r_bf = bcp.tile([128, ROWS], bf16, tag="bc", name="r_bf")
            nc.scalar.activation(r_bf[:], r_ps[:], AF.Copy)
            for kt in range(NT):
                t = tmp.tile([128, ROWS], bf16, tag="t2", name="lnt")
                nc.vector.tensor_tensor(t[:], xT_sb[:, kt], m_bf[:], OP.subtract)
                nc.vector.tensor_tensor(h_sb[:, kt], t[:], r_bf[:], OP.mult)

        # ============ QKV projections (fp8 DoubleRow) + RoPE + head projections ===
        # DoubleRow outputs are limited to psum partitions 0-63 (PE col tile
        # position must be 0), so each psum tile packs two 64-feature chunks
        # side by side; Act copies restore the 128-partition SBUF layout (and
        # apply bias / descale); RoPE then runs as all-SBUF bf16 TT ops.
        kb = acts.tile([128, NT, ROWS], bf16, tag="act16", name="kb")
        qb = acts.tile([128, NT, ROWS], bf16, tag="act16", name="qb")
        krot = acts.tile([128, NT, ROWS], fp8, tag="act8", name="krot")
        vbf = acts.tile([128, NT, ROWS], fp8, tag="act8", name="vbf")
        qrot = acts.tile([128, NT, ROWS], fp8, tag="act8", name="qrot")
        keyT_loc = const.tile([128, 2, ROWS], bf16)
        val_loc = const.tile([128, 4, HK], bf16)
        qhT = const.tile([128, 2, ROWS], bf16)
        in_cc = dram.tile([2 * HK * ROWS], bf16)   # keyh^T block then valh block
        out_cc = dram.tile([8 * HK * ROWS], bf16)

        def proj_pairs(pool, wname, have_bias, bias_row, dest):
            # h @ w -> dest [128, NT, ROWS]; feature-chunk pair (j, j+8) of 64
            # partitions shares one psum tile, slots land at (kt, kt+4).
            for j in range(NT):
                p = pool.tile([64, 2, ROWS], f32, tag="mm", name=f"{wname}p{j}")
                for sl in range(2):
                    c = j + 8 * sl
                    for kp in range(4):
                        nc.tensor.matmul(p[:, sl], w_sb[wname][:, 2 * kp:2 * kp + 2, 64 * c:64 * c + 64],
                                         h_sb[:, 2 * kp:2 * kp + 2],
                                         start=(kp == 0), stop=(kp == 3), perf_mode=DR)
                pp, kt = 64 * (j % 2), j // 2
                if have_bias:
                    for sl in range(2):
                        nc.scalar.activation(dest[pp:pp + 64, kt + 4 * sl], p[:, sl],
                                             AF.Identity, bias=bias_sb[pp:pp + 64, bias_row, kt + 4 * sl:kt + 4 * sl + 1])
                else:
                    nc.scalar.activation(dest[pp:pp + 64, kt:kt + 5:4, :], p[:], AF.Copy)

        def rope(src, out_sb):
            # src [128, NT, ROWS] bf16 (all-SBUF); out fp8
            for j in range(4):
                c = cs_sb[:, j, 0]
                sn = cs_sb[:, j, 1]
                t1 = tmp.tile([128, ROWS], bf16, tag="t2", name="r1")
                nc.vector.tensor_tensor(t1[:], src[:, j], c, OP.mult)
                t2 = tmp.tile([128, ROWS], bf16, tag="t2", name="r2")
                nc.vector.tensor_tensor(t2[:], src[:, j + 4], sn, OP.mult)
                t3 = tmp.tile([128, ROWS], bf16, tag="t2", name="r3")
                nc.vector.tensor_tensor(t3[:], src[:, j], sn, OP.mult)
                t4 = tmp.tile([128, ROWS], bf16, tag="t2", name="r4")
                nc.vector.tensor_tensor(t4[:], src[:, j + 4], c, OP.mult)
                nc.gpsimd.tensor_tensor(out_sb[:, j], t1[:], t2[:], OP.subtract)
                nc.gpsimd.tensor_tensor(out_sb[:, j + 4], t3[:], t4[:], OP.add)

        def head4(pool, wsb, rhs, dest, have_bias, bias_base):
            # rhs @ wsb -> 4 chunks of 64 partitions over dest [128, 2, ROWS];
            # pairs (0,2) and (1,3) share a psum tile (slots = mt2 0,1).
            for c0 in range(2):
                p = pool.tile([64, 2, ROWS], f32, tag="mm", name=f"hd{bias_base}_{c0}")
                for sl in range(2):
                    c = c0 + 2 * sl
                    for kp in range(4):
                        nc.tensor.matmul(p[:, sl], wsb[:, 2 * kp:2 * kp + 2, 64 * c:64 * c + 64],
                                         rhs[:, 2 * kp:2 * kp + 2],
                                         start=(kp == 0), stop=(kp == 3), perf_mode=DR)
                pp = 64 * (c0 % 2)
                if have_bias:
                    for sl in range(2):
                        nc.scalar.activation(dest[pp:pp + 64, sl], p[:, sl], AF.Identity,
                                             bias=bhc[pp:pp + 64, bias_base + sl:bias_base + sl + 1])
                else:
                    nc.scalar.activation(dest[pp:pp + 64, :, :], p[:], AF.Copy)

        with tc.tile_pool(name="ps_mm", bufs=3, space="PSUM") as ps_mm, \
             tc.tile_pool(name="ps_hv", bufs=2, space="PSUM") as ps_hv:
            proj_pairs(ps_mm, "w2", flags["b2"], 1, kb)
            proj_pairs(ps_mm, "w3", flags["b3"], 2, vbf)
            rope(kb, krot)
            # keyh^T = (v @ k_W [+ bk])^T
            head4(ps_mm, wk_sb, vbf, keyT_loc, flags["bk"], 2)
            # valh = (k_rot @ v_W [+ bv]) * 2^-5: row-chunk pairs (c, c+2)
            for pi in range(4):
                c0 = (pi // 2) * 4 + (pi % 2)
                p = ps_hv.tile([64, 2, HK], f32, tag="hv", name=f"vh{pi}")
                for sl in range(2):
                    c = c0 + 2 * sl
                    for kp in range(4):
                        nc.tensor.matmul(p[:, sl], krot[:, 2 * kp:2 * kp + 2, 64 * c:64 * c + 64],
                                         wv_sb[:, 2 * kp:2 * kp + 2],
                                         start=(kp == 0), stop=(kp == 3), perf_mode=DR)
                pp = 64 * (c0 % 2)
                st0 = c0 // 2
                if flags["bv"]:
                    for sl in range(2):
                        nc.vector.scalar_tensor_tensor(val_loc[pp:pp + 64, st0 + sl], p[:, sl], 2.0 ** -5,
                                                       bvb[pp:pp + 64, :], OP.mult, OP.add)
                else:
                    nc.scalar.activation(val_loc[pp:pp + 64, st0:st0 + 2, :], p[:], AF.Copy, scale=2.0 ** -5)
            # ship local kv and gather
            nc.sync.dma_start(in_cc[0:HK * ROWS].rearrange("(o p f) -> p o f", p=128, f=ROWS), keyT_loc[:])
            nc.sync.dma_start(in_cc[HK * ROWS:2 * HK * ROWS].rearrange("(st p c) -> p st c", p=128, c=HK), val_loc[:])
            nc.gpsimd.collective_compute(
                "AllGather", mybir.AluOpType.bypass,
                replica_groups=[[0, 1, 2, 3], [4, 5, 6, 7]],
                ins=[in_cc[:].opt()], outs=[out_cc[:].opt()],
            )
            # q side (overlaps the collective)
            proj_pairs(ps_mm, "w1", flags["b1"], 0, qb)
            rope(qb, qrot)
            head4(ps_mm, wq_sb, qrot, qhT, flags["bq"], 0)

        # gathered kv into SBUF; val augmented with ones column (value 0.5) per head
        keyT_all = const.tile([128, 8, ROWS], bf16)    # [:, 2*cb+G, s_local]
        val_aug = const.tile([128, 16, 8, 33], bf16)   # [:, 4*cb+lt, h, c|32=ones]
        nc.vector.memset(val_aug[:, :, :, 32:33], 0.5)
        BLK = 2 * HK * ROWS
        for cb in range(4):
            nc.sync.dma_start(keyT_all[:, 2 * cb:2 * cb + 2],
                              out_cc[cb * BLK:cb * BLK + HK * ROWS].rearrange("(o p f) -> p o f", p=128, f=ROWS))
            for st in range(4):
                base = cb * BLK + HK * ROWS + st * 128 * HK
                nc.sync.dma_start(val_aug[:, 4 * cb + st, :, 0:32],
                                  out_cc[base:base + 128 * HK].rearrange(
                                      "(p h c) -> p h c", p=128, h=H, c=K))

        # ================= attention =================
        # ao_ps[qt]: [128 q, 8 heads x 33] accumulated over stages; col 32 of each
        # head block is the softmax denominator (ones column).
        aoT = const.tile([128, 2, ROWS], bf16)
        x1 = acts.tile([128, NT, ROWS], bf16, tag="act16", name="x1")
        with tc.tile_pool(name="ps_ao", bufs=4, space="PSUM") as ps_ao:
            ao_ps = []
            for qt in range(4):
                ao_ps.append(ps_ao.tile([128, 8, 33], f32, tag="ao", name=f"ao{qt}"))
            with tc.tile_pool(name="ps_sc", bufs=2, space="PSUM") as ps_sc:
                E_all = {}
                for G in range(2):
                    for stg in range(16):
                        cb, lt = stg // 4, stg % 4
                        for pr in range(2):
                            sc = ps_sc.tile([128, 2, ROWS], f32, tag="sc", name=f"sc{G}_{stg}_{pr}")
                            for j in range(2):
                                hh = 2 * pr + j
                                nc.tensor.matmul(sc[:, j],
                                                 keyT_all[32 * hh:32 * hh + 32, 2 * cb + G, lt * 128:(lt + 1) * 128],
                                                 qhT[32 * hh:32 * hh + 32, G],
                                                 start=True, stop=True, tile_position=(32 * hh, 0))
                            E = epool.tile([128, 2, ROWS], fp8, tag="E", name=f"E{G}_{stg}_{pr}")
                            nc.scalar.activation(E[:], sc[:], AF.Exp, scale=EXP_SCALE)
                            E_all[(G, stg, pr)] = E
                    # sequential per-(qt,head) accumulation chains (start marks the
                    # whole psum bank pending-zero, so chains must not interleave
                    # within a bank)
                    for qt in range(4):
                        for ph in range(4):
                            hh = 4 * G + ph
                            for stg in range(16):
                                E = E_all[(G, stg, ph // 2)]
                                nc.tensor.matmul(
                                    ao_ps[qt][:, hh, :],
                                    E[:, ph % 2, qt * 128:(qt + 1) * 128],
                                    val_aug[:, stg, hh, :],
                                    start=(stg == 0), stop=(stg == 15))
            # normalize by the ones-column sums, per q-tile / head
            with tc.tile_pool(name="ps_tr", bufs=2, space="PSUM") as ps_tr:
                for qt in range(4):
                    rdn = rows.tile([128, 8, 1], f32, tag="rdn", name=f"rdn{qt}")
                    nc.vector.reciprocal(rdn[:], ao_ps[qt][:, :, 32:33])
                    aon = aonp.tile([128, H, 32], bf16, tag="aon", name=f"aon{qt}")
                    for hh in range(H):
                        nc.scalar.activation(aon[:, hh], ao_ps[qt][:, hh, 0:32], AF.Copy, scale=rdn[:, hh])
                    # transpose [128 q, 256 hd] -> aoT [256 hd, 512 q]
                    for hb in range(2):
                        ptr = ps_tr.tile([128, 128], bf16, tag="tr", name=f"tr{qt}_{hb}")
                        nc.tensor.transpose(ptr[:], aon[:, 4 * hb:4 * hb + 4], ident[:])
                        nc.scalar.activation(aoT[:, hb, qt * 128:(qt + 1) * 128], ptr[:], AF.Copy)

        # o_proj + residual (o_b folded into gate/up biases on host)
        with tc.tile_pool(name="ps_o", bufs=4, space="PSUM") as ps_o:
            for mt in range(NT):
                p = ps_o.tile([128, ROWS], f32, tag="o", name=f"ops{mt}")
                for G in range(2):
                    nc.tensor.matmul(p[:], wo_sb[:, G, mt * 128:(mt + 1) * 128], aoT[:, G],
                                     start=(G == 0), stop=(G == 1))
                nc.vector.tensor_tensor(x1[:, mt], p[:], xT_sb[:, mt], OP.add)

        # ============ SwiGLU (bf16) + LN2 + out ============
        sg = acts.tile([128, NT, ROWS], bf16, tag="act16", name="sg")
        mm_sb = acts.tile([128, NT, ROWS], bf16, tag="act16", name="mm_sb")
        f_sb = acts.tile([128, NT, ROWS], bf16, tag="act16", name="f_sb")
        for nm in ("wg", "wu"):
            w_sb[nm] = wbg.tile([128, NT, D], bf16, tag="Wbig", name=f"W_{nm}")
            nc.sync.dma_start(w_sb[nm][:], wbig[nm].rearrange("(o p) c -> p o c", p=128))

        def big_mm16(psum_t, wname, mt, rhs_sb):
            for kt in range(NT):
                nc.tensor.matmul(psum_t[:], w_sb[wname][:, kt, mt * 128:(mt + 1) * 128],
                                 rhs_sb[:, kt], start=(kt == 0), stop=(kt == NT - 1))

        with tc.tile_pool(name="ps_mm2", bufs=8, space="PSUM") as ps2:
            for mt in range(NT):
                p = ps2.tile([128, ROWS], f32, tag="mm2", name=f"gps{mt}")
                big_mm16(p, "wg", mt, x1)
                nc.scalar.activation(sg[:, mt], p[:], AF.Silu, bias=col(3, mt))
            w_sb["wd"] = wbg.tile([128, NT, D], bf16, tag="Wbig", name="W_wd")
            nc.sync.dma_start(w_sb["wd"][:], wbig["wd"].rearrange("(o p) c -> p o c", p=128))
            for mt in range(NT):
                p = ps2.tile([128, ROWS], f32, tag="mm2", name=f"ups{mt}")
                big_mm16(p, "wu", mt, x1)
                if flags["bu"]:
                    nc.vector.scalar_tensor_tensor(mm_sb[:, mt], p[:], col(4, mt), sg[:, mt], OP.add, OP.mult)
                else:
                    nc.vector.tensor_tensor(mm_sb[:, mt], p[:], sg[:, mt], OP.mult)

        with tc.tile_pool(name="ps_mm3", bufs=6, space="PSUM") as ps3, \
             tc.tile_pool(name="ps_st2", bufs=2, space="PSUM") as ps_st2:
            sum2 = ps_st2.tile([1, ROWS], f32, tag="st2", name="sum2")
            sumsq2 = ps_st2.tile([1, ROWS], f32, tag="st2", name="sumsq2")
            for mt in range(NT):
                p = ps3.tile([128, ROWS], f32, tag="mm3", name=f"dps{mt}")
                big_mm16(p, "wd", mt, mm_sb)
                if flags["bd"]:
                    nc.vector.tensor_scalar_add(f_sb[:, mt], p[:], col(5, mt))
                else:
                    nc.vector.tensor_copy(out=f_sb[:, mt], in_=p[:])
                sqf = tmp.tile([128, ROWS], bf16, tag="t2", name="sqf")
                nc.vector.tensor_tensor(sqf[:], f_sb[:, mt], f_sb[:, mt], OP.mult)
                nc.tensor.matmul(sum2[:], invd[:], f_sb[:, mt], start=(mt == 0), stop=(mt == NT - 1))
                nc.tensor.matmul(sumsq2[:], invd[:], sqf[:], start=(mt == 0), stop=(mt == NT - 1))
            mean2 = rows.tile([1, ROWS], f32, tag="row", name="mean2")
            nc.vector.tensor_copy(out=mean2[:], in_=sum2[:])
            msq2 = rows.tile([1, ROWS], f32, tag="row", name="msq2")
            nc.vector.tensor_tensor(msq2[:], mean2[:], mean2[:], OP.mult)
            var2 = rows.tile([1, ROWS], f32, tag="row", name="var2")
            nc.vector.tensor_tensor(var2[:], sumsq2[:], msq2[:], OP.subtract)
            std2 = rows.tile([1, ROWS], f32, tag="row", name="std2")
            nc.scalar.activation(std2[:], var2[:], AF.Sqrt, bias=eps_sb[:])
            rstd2 = rows.tile([1, ROWS], f32, tag="row", name="rstd2")
            nc.vector.reciprocal(rstd2[:], std2[:])
            m2_ps = ps_st2.tile([128, ROWS], f32, tag="st2", name="m2_ps")
            nc.tensor.matmul(m2_ps[:], ones128[:], mean2[:], start=True, stop=True)
            r2_ps = ps_st2.tile([128, ROWS], f32, tag="st2", name="r2_ps")
            nc.tensor.matmul(r2_ps[:], ones128[:], rstd2[:], start=True, stop=True)
            m2_bf = bcp.tile([128, ROWS], bf16, tag="bc", name="m2_bf")
            nc.scalar.activation(m2_bf[:], m2_ps[:], AF.Copy)
            r2_bf = bcp.tile([128, ROWS], bf16, tag="bc", name="r2_bf")
            nc.scalar.activation(r2_bf[:], r2_ps[:], AF.Copy)
            for mt in range(NT):
                t1 = tmp.tile([128, ROWS], bf16, tag="t2", name="o1")
                nc.vector.tensor_tensor(t1[:], f_sb[:, mt], m2_bf[:], OP.subtract)
                t2 = tmp.tile([128, ROWS], bf16, tag="t2", name="o2")
                nc.vector.tensor_tensor(t2[:], t1[:], r2_bf[:], OP.mult)
                t3 = tmp.tile([128, ROWS], bf16, tag="t2", name="o3")
                if flags["g2"]:
                    nc.vector.tensor_scalar_mul(t3[:], t2[:], col(6, mt))
                    t2 = t3
                    t3 = tmp.tile([128, ROWS], bf16, tag="t2", name="o3b")
                if flags["bln2"]:
                    nc.vector.scalar_tensor_tensor(t3[:], t2[:], col(7, mt), f_sb[:, mt], OP.add, OP.add)
                else:
                    nc.vector.tensor_tensor(t3[:], t2[:], f_sb[:, mt], OP.add)
                nc.sync.dma_start(outT[mt * 128:(mt + 1) * 128, :], t3[:])
        ctx.close()

    nc.compile()
    return nc


def _prep_inputs(inputs):
    x = np.asarray(inputs["x"], np.float32)
    g1 = np.asarray(inputs["ln1_g"], np.float32)
    b1 = np.asarray(inputs["ln1_b"], np.float32)
    sc = 1.0 / math.sqrt(K)
    S1, S1Q, S1K, S1V = 2.0 ** 5, 2.0 ** 7, 2.0 ** 5, 2.0 ** 5

    def fold(Wn, bn, s):
        W = np.asarray(inputs[Wn], np.float32)
        b = np.asarray(inputs[bn], np.float32)
        return (g1[:, None] * W * s).astype(FP8), ((b + b1 @ W) * s).astype(np.float32)

    w1, bw1 = fold("w1_W", "w1_b", S1)
    w2, bw2 = fold("w2_W", "w2_b", S1)
    w3, bw3 = fold("w3_W", "w3_b", S1)
    # head projections: q scaled by sc*2^7, k by 2^5, v by 2^5 (descaled at copy)
    wqv = (np.asarray(inputs["q_W"], np.float32).reshape(D, HK) * sc * S1Q).astype(FP8)
    bq = (np.asarray(inputs["q_b"], np.float32).reshape(HK) * sc * S1 * S1Q).astype(np.float32)
    wkv = (np.asarray(inputs["k_W"], np.float32).reshape(D, HK) * S1K).astype(FP8)
    bk = (np.asarray(inputs["k_b"], np.float32).reshape(HK) * S1 * S1K).astype(np.float32)
    wvv = (np.asarray(inputs["v_W"], np.float32).reshape(D, HK) * S1V).astype(FP8)
    bv = (np.asarray(inputs["v_b"], np.float32).reshape(HK) * 2.0 ** 5).astype(np.float32)
    # ao_norm carries 2^6; fold 2^-6 into o_W
    wov = (np.asarray(inputs["o_W"], np.float32).reshape(HK, D) * 2.0 ** -6).astype(BF16)
    o_b = np.asarray(inputs["o_b"], np.float32)
    gW = np.asarray(inputs["gate_W"], np.float32)
    uW = np.asarray(inputs["up_W"], np.float32)
    gb_eff = np.asarray(inputs["gate_b"], np.float32) + o_b @ gW
    ub_eff = np.asarray(inputs["up_b"], np.float32) + o_b @ uW

    bcols = np.zeros((10, D), np.float32)
    bcols[0] = bw1
    bcols[1] = bw2
    bcols[2] = bw3
    bcols[3] = gb_eff
    bcols[4] = ub_eff
    bcols[5] = np.asarray(inputs["down_b"], np.float32)
    bcols[6] = np.asarray(inputs["ln2_g"], np.float32)
    bcols[7] = np.asarray(inputs["ln2_b"], np.float32)
    bcols[8, 0:HK] = bq
    bcols[8, HK:2 * HK] = bk
    bcols[8, 2 * HK:3 * HK] = bv

    flags = {
        "b1": bw1.any(), "b2": bw2.any(), "b3": bw3.any(),
        "bq": bq.any(), "bk": bk.any(), "bv": bv.any(),
        "bu": ub_eff.any(), "bd": bcols[5].any(),
        "g2": not np.allclose(bcols[6], 1.0), "bln2": bcols[7].any(),
    }
    flags = {k: bool(v) for k, v in flags.items()}

    wgv = gW.astype(BF16)
    wuv = uW.astype(BF16)
    wdv = np.asarray(inputs["down_W"], np.float32).astype(BF16)

    pos = np.arange(S, dtype=np.float32)
    freq = np.power(10000.0, -np.arange(D // 2, dtype=np.float32) / (D // 2))
    ang = pos[:, None] * freq[None, :]  # [S, 512]
    cosA = np.cos(ang).astype(np.float32)
    sinA = np.sin(ang).astype(np.float32)

    ident = np.eye(128, dtype=np.float32).astype(BF16)

    in_maps = []
    for c in range(N_CORES):
        b = c // 4
        j = c % 4
        sl = slice(ROWS * j, ROWS * (j + 1))
        # cossin [128, 4, 2, ROWS]: [p, o, {cos,sin}, row] with feature f = o*128+p
        cs = np.empty((128, 4, 2, ROWS), np.float32)
        cT = cosA[sl, :].T.reshape(4, 128, ROWS)
        sT = sinA[sl, :].T.reshape(4, 128, ROWS)
        cs[:, :, 0, :] = cT.transpose(1, 0, 2)
        cs[:, :, 1, :] = sT.transpose(1, 0, 2)
        m = {
            "xT": np.ascontiguousarray(x[b, sl, :].T).astype(BF16),
            "w1": w1, "w2": w2, "w3": w3,
            "wg": wgv, "wu": wuv, "wd": wdv,
            "wq": wqv, "wk": wkv, "wv": wvv, "wo": wov,
            "bcols": bcols,
            "cossin": cs.astype(FP8),
            "identD": ident,
        }
        in_maps.append(m)
    return in_maps, flags


def kernel(**inputs):
    from concourse.bass_utils import run_bass_kernel_spmd

    in_maps, flags = _prep_inputs(inputs)
    key = tuple(sorted(flags.items()))
    if key not in _CACHE:
        _CACHE[key] = _build_program(flags)
    nc = _CACHE[key]
    res = run_bass_kernel_spmd(nc, in_maps, list(range(N_CORES)))
    out = np.empty((B, S, D), np.float32)
    for c in range(N_CORES):
        b = c // 4
        j = c % 4
        out[b, ROWS * j:ROWS * (j + 1), :] = res.results[c]["outT"].astype(np.float32).T
    return out
